# revision 1
# baseline (speedup 1.0000x reference)
"""Trainium2 Bass kernel for a 2-layer GCN (BiomassGNN) on 8 NeuronCores.

Strategy (edge partition by destination, per the sharding hint):
  - Host: add self-loops, compute dinv = 1/sqrt(deg), assign each edge to the
    core owning its dst node (12500 nodes/core).  Within a core, edges are
    bucketed by (src chunk of 25000 rows, dst block of 128 rows); buckets are
    padded to sizes common across all 8 cores so ONE static program serves
    every core (SPMD).
  - Device, per layer: every core computes its shard of the scaled feature
    table T = (h @ W) * dinv[node], AllGathers the full [N, 64] fp32 table in
    DRAM, then dma_gather's the 64-float rows for its edges (descriptor-bound),
    builds one-hot selection tiles on the vector engine
    (is_equal(iota, dst_rel)), and aggregates per dst block with TensorE
    matmuls accumulating in PSUM; per-bucket partials are accumulated in SBUF.
    BatchNorm+bias+ReLU(+residual) are folded into a per-block epilogue.
  - Readout: mean-pool via one-hot(batch) matmuls -> [64 feat, 64 graph]
    partials -> AllReduce -> small MLP computed redundantly on every core.
"""

from contextlib import ExitStack

import numpy as np

import concourse.bass as bass
import concourse.bacc as bacc
import concourse.mybir as mybir
import concourse.tile as tile

F32 = mybir.dt.float32
I16 = mybir.dt.int16
ALU = mybir.AluOpType

NCORES = 8


class Cfg:
    def __init__(self, N=100000, E=1200000, G=64, chunk=25000, win=8192,
                 min_bucket=256, single_packet=False):
        self.N, self.E, self.G = N, E, G
        self.DIN, self.H, self.L = 7, 64, 2
        self.NPC = N // NCORES              # nodes per core
        self.NB = (self.NPC + 127) // 128   # dst blocks per core
        self.TAILP = self.NPC - (self.NB - 1) * 128  # partitions in last block
        self.CHUNK = chunk                  # src rows per gather chunk
        self.NCH = N // chunk               # number of chunks
        assert N % NCORES == 0 and N % chunk == 0 and chunk <= 32768
        self.WIN = win                      # edges per gather window
        assert win % 128 == 0
        self.MIN_BUCKET = min_bucket
        self.SINGLE_PACKET = single_packet


# ----------------------------------------------------------------------------
# Host-side graph preprocessing
# ----------------------------------------------------------------------------

class Prep:
    """Static-shape edge layout + per-core data arrays."""

    def __init__(self, cfg: Cfg, edge_index: np.ndarray, batch: np.ndarray):
        c = cfg
        N = c.N
        src = np.concatenate([edge_index[0].astype(np.int64),
                              np.arange(N, dtype=np.int64)])
        dst = np.concatenate([edge_index[1].astype(np.int64),
                              np.arange(N, dtype=np.int64)])
        deg = np.bincount(dst, minlength=N).astype(np.float32)
        self.dinv = (1.0 / np.sqrt(deg)).astype(np.float32)

        core = dst // c.NPC
        dstl = dst - core * c.NPC
        blk = dstl // 128
        ch = src // c.CHUNK

        key = (core * c.NCH + ch) * c.NB + blk
        counts = np.bincount(key, minlength=NCORES * c.NCH * c.NB)
        counts = counts.reshape(NCORES, c.NCH, c.NB)
        common = counts.max(axis=0)                       # [NCH, NB]
        common = np.maximum(common, c.MIN_BUCKET)
        for cc in range(c.NCH):                           # align chunk totals
            common[cc, c.NB - 1] += (-common[cc].sum()) % 128
        self.common = common
        self.TOT = int(common.sum())
        assert self.TOT % 128 == 0

        # bucket start offsets in the edge stream (same for every core)
        off = np.zeros((c.NCH, c.NB), dtype=np.int64)
        pos = 0
        self.chunk_range = []
        for cc in range(c.NCH):
            s0 = pos
            for b in range(c.NB):
                off[cc, b] = pos
                pos += common[cc, b]
            self.chunk_range.append((s0, pos))
        self.off = off

        # ---- static slice metadata --------------------------------------
        nsl = self.TOT // 128
        self.slice_chunk = np.zeros(nsl, dtype=np.int64)
        self.slice_base = np.zeros(nsl, dtype=np.int64)    # base block
        self.slice_span = np.zeros(nsl, dtype=bool)
        ends = off + common                                # bucket end offsets
        for cc in range(c.NCH):
            s0, s1 = self.chunk_range[cc]
            b = 0
            for s in range(s0 // 128, s1 // 128):
                start = s * 128
                while ends[cc, b] <= start:
                    b += 1
                self.slice_chunk[s] = cc
                self.slice_base[s] = b
                self.slice_span[s] = (start + 128 > ends[cc, b])
        # per-bucket list of (slice, which) in stream order
        self.bucket_mm = {}
        for s in range(nsl):
            cc, b = int(self.slice_chunk[s]), int(self.slice_base[s])
            self.bucket_mm.setdefault((cc, b), []).append((s, 1))
            if self.slice_span[s]:
                self.bucket_mm.setdefault((cc, b + 1), []).append((s, 2))
        for kk in self.bucket_mm:
            self.bucket_mm[kk].sort(key=lambda t: t[0])

        # ---- per-core data arrays ---------------------------------------
        order = np.lexsort((dst, ch, core))
        src_s, dst_s = src[order], dst[order]
        core_s, ch_s, blk_s = core[order], ch[order], blk[order]
        dstl_s = dstl[order]

        self.gidx = np.zeros((NCORES, self.TOT), dtype=np.int16)
        self.dstrel = np.full((NCORES, self.TOT), -65536.0, dtype=np.float32)
        # per-(core,chunk,block) segment boundaries in the sorted arrays
        seg = np.zeros((NCORES, c.NCH, c.NB + 1), dtype=np.int64)
        flat_counts = counts.reshape(-1)
        cum = np.concatenate([[0], np.cumsum(flat_counts)])
        for k in range(NCORES):
            for cc in range(c.NCH):
                for b in range(c.NB):
                    i = (k * c.NCH + cc) * c.NB + b
                    seg[k, cc, b] = cum[i]
                    seg[k, cc, b + 1] = cum[i + 1]
        slice_of = np.arange(self.TOT) // 128
        base_of_pos = self.slice_base[slice_of]            # [TOT]
        for k in range(NCORES):
            for cc in range(c.NCH):
                for b in range(c.NB):
                    n = int(counts[k, cc, b])
                    if n == 0:
                        continue
                    a0 = int(seg[k, cc, b])
                    o0 = int(off[cc, b])
                    self.gidx[k, o0:o0 + n] = (src_s[a0:a0 + n] % c.CHUNK
                                               ).astype(np.int16)
                    self.dstrel[k, o0:o0 + n] = (
                        dstl_s[a0:a0 + n] - 128.0 * base_of_pos[o0:o0 + n]
                    ).astype(np.float32)

        # batch ids per node, wrapped [128, NB] per core; -1 padding
        self.batch_w = np.full((NCORES, 128, c.NB), -1.0, dtype=np.float32)
        self.dinv_w = np.zeros((NCORES, 128, c.NB), dtype=np.float32)
        for k in range(NCORES):
            seg_b = batch[k * c.NPC:(k + 1) * c.NPC].astype(np.float32)
            seg_d = self.dinv[k * c.NPC:(k + 1) * c.NPC]
            pad = c.NB * 128 - c.NPC
            seg_b = np.concatenate([seg_b, np.full(pad, -1.0, np.float32)])
            seg_d = np.concatenate([seg_d, np.zeros(pad, np.float32)])
            self.batch_w[k] = seg_b.reshape(c.NB, 128).T
            self.dinv_w[k] = seg_d.reshape(c.NB, 128).T

        # graph-size reciprocal (guard cnt==0 like the reference)
        cnt = np.bincount(batch.astype(np.int64), minlength=c.G
                          ).astype(np.float32)
        self.invcnt = (1.0 / np.maximum(cnt, 1.0)).astype(np.float32)

    def wrap16(self, arr):
        """[TOT] -> [128, TOT//16] int16 layout for dma_gather indexes."""
        t = arr.reshape(-1, 16).T                  # [16, TOT//16]
        return np.tile(t, (8, 1)).copy()           # replicate for 8 Q7 cores

    def wrap128(self, arr):
        """[TOT] -> [128, TOT//128] edge-major (partition = e%128)."""
        return arr.reshape(-1, 128).T.copy()


# ----------------------------------------------------------------------------
# Kernel builder
# ----------------------------------------------------------------------------

def build_kernel(cfg: Cfg, prep: Prep, mode="full", repeat=1):
    c = cfg
    H = c.H
    nc = bacc.Bacc(None, num_devices=NCORES)

    def din(name, shape, dtype=F32):
        return nc.dram_tensor(name, list(shape), dtype, kind="ExternalInput")

    xT = din("xT", (c.DIN, c.NPC))
    W_in = din("W_in", (c.DIN, H))
    bin_t = din("bin_t", (128, H))
    convW = [din(f"convW{i}", (H, H)) for i in range(2)]
    s_t = [din(f"s_t{i}", (128, H)) for i in range(2)]
    t_t = [din(f"t_t{i}", (128, H)) for i in range(2)]
    dinv_in = din("dinv_w", (128, c.NB))
    batch_in = din("batch_w", (128, c.NB))
    iota1_in = din("iota1", (128, 128))
    iota2_in = din("iota2", (128, 128))
    iotaG_in = din("iotaG", (128, c.G))
    ident_in = din("ident", (128, 128))
    invcnt_in = din("invcnt_t", (c.G, c.G))
    mW1 = din("mW1", (H, H // 2))
    mb1 = din("mb1", (H // 2, 1))
    mW2 = din("mW2", (H // 2, 1))
    mb2 = din("mb2", (1, 1))
    gidx_in = din("gidx", (128, prep.TOT // 16), I16)
    dstrel_in = din("dstrel", (128, prep.TOT // 128))

    out_t = nc.dram_tensor("out", [c.G, 1], F32, kind="ExternalOutput")

    with tile.TileContext(nc, num_cores=NCORES) as tc, ExitStack() as ctx:
        P = {}
        for name, bufs, kw in [
            ("const", 1, {}),
            ("persist", 1, {}),
            ("msgs", 3, {}),
            ("sel", 4, {}),
            ("work", 3, {}),
            ("tblout", 2, {}),
            ("psA", 3, dict(space="PSUM")),   # aggregation
            ("psT", 2, dict(space="PSUM")),   # transposes
            ("psW", 2, dict(space="PSUM")),   # weight matmuls / mlp
            ("psP", 1, dict(space="PSUM")),   # pooling accumulator
            ("dram", 1, dict(space="DRAM")),
        ]:
            P[name] = ctx.enter_context(tc.tile_pool(name=name, bufs=bufs, **kw))

        def load_const(tensor, shape, dtype=F32):
            t = P["const"].tile(shape, dtype, name=f"c_{tensor.name}",
                                tag=f"c_{tensor.name}")
            nc.sync.dma_start(t[:], tensor.ap())
            return t

        xT_sb = P["const"].tile([c.DIN, c.NB * 128], F32, name="c_xT",
                                tag="c_xT")
        if c.NB * 128 > c.NPC:
            nc.vector.memset(xT_sb[:, c.NPC:], 0.0)
        nc.sync.dma_start(xT_sb[:, :c.NPC], xT.ap())
        Win_sb = load_const(W_in, (c.DIN, H))
        binT_sb = load_const(bin_t, (128, H))
        convW_sb = [load_const(convW[i], (H, H)) for i in range(2)]
        sT_sb = [load_const(s_t[i], (128, H)) for i in range(2)]
        tT_sb = [load_const(t_t[i], (128, H)) for i in range(2)]
        dinv_sb = load_const(dinv_in, (128, c.NB))
        batch_sb = load_const(batch_in, (128, c.NB))
        iota1_sb = load_const(iota1_in, (128, 128))
        iota2_sb = load_const(iota2_in, (128, 128))
        iotaG_sb = load_const(iotaG_in, (128, c.G))
        ident_sb = load_const(ident_in, (128, 128))
        invcnt_sb = load_const(invcnt_in, (c.G, c.G))
        mW1_sb = load_const(mW1, (H, H // 2))
        mb1_sb = load_const(mb1, (H // 2, 1))
        mW2_sb = load_const(mW2, (H // 2, 1))
        mb2_sb = load_const(mb2, (1, 1))
        gidx_sb = P["persist"].tile([128, prep.TOT // 16], I16,
                                    name="gidx_sb", tag="gidx_sb")
        nc.sync.dma_start(gidx_sb[:], gidx_in.ap())
        dstrel_sb = P["persist"].tile([128, prep.TOT // 128], F32,
                                      name="dstrel_sb", tag="dstrel_sb")
        nc.sync.dma_start(dstrel_sb[:], dstrel_in.ap())

        acc = P["persist"].tile([128, c.NB * H], F32, name="acc", tag="acc")
        h1 = P["persist"].tile([128, c.NB * H], F32, name="h1", tag="h1")

        bounce = P["dram"].tile([c.NPC, H], F32, name="bounce", tag="bounce")
        ar_in = P["dram"].tile([H, c.G], F32, name="ar_in", tag="ar_in")

        rg = [list(range(NCORES))]

        # ------------------------------------------------------------------
        def build_table(get_hfm_block, W_sb, T_full):
            """table[n,:] = (h @ W)[n,:] * dinv[n]; write shard + AllGather.

            get_hfm_block(b) -> SBUF tile [H, 128] holding h^T for block b.
            """
            GRP = 16
            for b0 in range(0, c.NB, GRP):
                b1 = min(b0 + GRP, c.NB)
                tbl = P["tblout"].tile([128, GRP * H], F32, tag="tbl")
                for b in range(b0, b1):
                    hfm = get_hfm_block(b)
                    ps = P["psW"].tile([128, H], F32, tag="wmm")
                    nc.tensor.matmul(ps[:], hfm[:], W_sb[:],
                                     start=True, stop=True)
                    nc.vector.tensor_scalar_mul(
                        tbl[:, (b - b0) * H:(b - b0 + 1) * H], ps[:],
                        dinv_sb[:, b:b + 1])
                # DMA the group to the bounce shard
                nfull = b1 - b0
                rows0 = b0 * 128
                if b1 == c.NB and c.TAILP < 128:
                    nfull -= 1
                if nfull > 0:
                    dview = bounce[rows0:rows0 + nfull * 128, :].rearrange(
                        "(b p) h -> p b h", p=128)
                    sview = tbl[:, :nfull * H].rearrange(
                        "p (b h) -> p b h", h=H)
                    nc.sync.dma_start(dview, sview)
                if b1 == c.NB and c.TAILP < 128:
                    rows1 = (c.NB - 1) * 128
                    nc.sync.dma_start(
                        bounce[rows1:rows1 + c.TAILP, :],
                        tbl[:c.TAILP, (c.NB - 1 - b0) * H:
                            (c.NB - b0) * H])
            if mode == "nocoll":
                nc.sync.dma_start(T_full[:c.NPC, :], bounce[:])
            else:
                nc.gpsimd.collective_compute(
                    "AllGather", ALU.bypass, replica_groups=rg,
                    ins=[bounce[:]], outs=[T_full[:]])

        # ------------------------------------------------------------------
        def hfm_from_h(h_tile):
            """Return fn(b) -> [H, 128] feature-major tile of h block b."""
            def get(b):
                pst = P["psT"].tile([H, 128], F32, tag="tp")
                nc.tensor.transpose(pst[:], h_tile[:, b * H:(b + 1) * H],
                                    ident_sb[:])
                hfm = P["work"].tile([H, 128], F32, tag="hfm")
                nc.vector.tensor_copy(hfm[:], pst[:])
                return hfm
            return get

        # ------------------------------------------------------------------
        def layer1_hfm(b):
            """h0^T block: relu(x @ W_in + b_in) for block b, transposed."""
            ps = P["psW"].tile([128, H], F32, tag="wmm")
            nc.tensor.matmul(ps[:], xT_sb[:, b * 128:(b + 1) * 128],
                             Win_sb[:], start=True, stop=True)
            h0 = P["work"].tile([128, H], F32, tag="h0")
            nc.vector.tensor_tensor(h0[:], ps[:], binT_sb[:], ALU.add)
            nc.vector.tensor_scalar_max(h0[:], h0[:], 0.0)
            pst = P["psT"].tile([H, 128], F32, tag="tp")
            nc.tensor.transpose(pst[:], h0[:], ident_sb[:])
            hfm = P["work"].tile([H, 128], F32, tag="hfm")
            nc.vector.tensor_copy(hfm[:], pst[:])
            return hfm

        # ------------------------------------------------------------------
        def aggregate(T_full):
            """Gather + one-hot matmul aggregation into `acc`."""
            if mode == "noagg":
                nc.vector.memset(acc[:], 0.001)
            # gather windows
            win_tiles = {}
            for cc in range(c.NCH):
                s0, s1 = prep.chunk_range[cc]
                w0 = s0
                while w0 < s1:
                    w = min(c.WIN, s1 - w0)
                    mt = P["msgs"].tile([128, (c.WIN // 128) * H], F32,
                                        tag="msg")
                    m3 = mt.rearrange("p (j h) -> p j h", h=H)
                    if mode == "nogather":
                        nc.vector.memset(mt[:, 0:1], 0.5)
                        win_tiles[w0] = (mt, w)
                        w0 += w
                        continue
                    nc.gpsimd.dma_gather(
                        m3[:, :w // 128, :],
                        T_full[cc * c.CHUNK:(cc + 1) * c.CHUNK, :],
                        gidx_sb[:, w0 // 16:(w0 + w) // 16],
                        w, w, H, single_packet=c.SINGLE_PACKET)
                    if mode == "noagg":
                        scr = P["sel"].tile([128, 1], F32, tag="scr")
                        nc.vector.tensor_copy(scr[:], mt[:, 0:1])
                    win_tiles[w0] = (mt, w)
                    w0 += w

            def msg_ap(s):
                pos = s * 128
                for w0, (mt, w) in win_tiles.items():
                    if w0 <= pos < w0 + w:
                        j = (pos - w0) // 128
                        return mt.rearrange("p (j h) -> p j h", h=H)[:, j, :]
                raise AssertionError

            if mode == "noagg":
                return
            const_sel = None
            if mode == "nosel":
                const_sel = P["persist"].tile([128, 128], F32,
                                              name="const_sel",
                                              tag="const_sel")
                nc.vector.memset(const_sel[:], 0.01)
            # per-bucket accumulation
            for cc in range(c.NCH):
                for b in range(c.NB):
                    mms = prep.bucket_mm.get((cc, b), [])
                    if not mms:
                        continue
                    ps = P["psA"].tile([128, H], F32, tag="agg")
                    for i, (s, which) in enumerate(mms):
                        if mode == "nosel":
                            sel = const_sel
                        else:
                            sel = P["sel"].tile([128, 128], F32, tag="sel")
                            iota = iota1_sb if which == 1 else iota2_sb
                            nc.vector.tensor_scalar(
                                sel[:], iota[:], dstrel_sb[:, s:s + 1], None,
                                ALU.is_equal)
                        nc.tensor.matmul(ps[:], sel[:], msg_ap(s),
                                         start=(i == 0),
                                         stop=(i == len(mms) - 1))
                    dstc = acc[:, b * H:(b + 1) * H]
                    if cc == 0:
                        nc.vector.tensor_copy(dstc, ps[:])
                    else:
                        nc.vector.tensor_tensor(dstc, dstc, ps[:], ALU.add)

        def epilogue(layer, b, pool_ps):
            """BN+bias+relu (+residual, +pool matmul on layer 1)."""
            sb = P["work"].tile([128, H], F32, tag="epi")
            nc.vector.scalar_tensor_tensor(
                sb[:], acc[:, b * H:(b + 1) * H], dinv_sb[:, b:b + 1],
                sT_sb[layer][:], ALU.mult, ALU.mult)
            nc.vector.tensor_tensor(sb[:], sb[:], tT_sb[layer][:], ALU.add)
            nc.vector.tensor_scalar_max(sb[:], sb[:], 0.0)
            if layer == 0:
                nc.vector.tensor_copy(h1[:, b * H:(b + 1) * H], sb[:])
                return None
            # layer 1: residual then pooling matmul
            nc.vector.tensor_tensor(sb[:], sb[:], h1[:, b * H:(b + 1) * H],
                                    ALU.add)
            selp = P["sel"].tile([128, c.G], F32, tag="selp")
            nc.vector.tensor_scalar(selp[:], iotaG_sb[:],
                                    batch_sb[:, b:b + 1], None, ALU.is_equal)
            nc.tensor.matmul(pool_ps[:], sb[:], selp[:],
                             start=(b == 0), stop=(b == c.NB - 1))
            return None

        # ==================================================================
        for rep in range(repeat):
            T_fulls = [P["dram"].tile([c.N, H], F32, addr_space="Shared",
                                      name=f"T_full{rep}_{i}",
                                      tag=f"T_full{rep}_{i}")
                       for i in range(2)]
            ar_out = P["dram"].tile([H, c.G], F32, addr_space="Shared",
                                    name=f"ar_out{rep}", tag=f"ar_out{rep}")
            pool_ps = P["psP"].tile([H, c.G], F32, name="pool_ps",
                                    tag="pool_ps")
            # Layer 0
            build_table(layer1_hfm, convW_sb[0], T_fulls[0])
            aggregate(T_fulls[0])
            for b in range(c.NB):
                epilogue(0, b, pool_ps)
            # Layer 1
            build_table(hfm_from_h(h1), convW_sb[1], T_fulls[1])
            aggregate(T_fulls[1])
            for b in range(c.NB):
                epilogue(1, b, pool_ps)

            # readout: pooled sums [H, G] -> AllReduce -> mean -> MLP
            pool_sb = P["work"].tile([H, c.G], F32, name="pool_sb",
                                     tag="pool_sb")
            nc.vector.tensor_copy(pool_sb[:], pool_ps[:])
            nc.sync.dma_start(ar_in[:], pool_sb[:])
            nc.gpsimd.collective_compute(
                "AllReduce", ALU.add, replica_groups=rg,
                ins=[ar_in[:]], outs=[ar_out[:]])
            pooled = P["work"].tile([H, c.G], F32, name="pooled",
                                    tag="pooled")
            nc.sync.dma_start(pooled[:], ar_out[:])
            nc.vector.tensor_tensor(pooled[:], pooled[:], invcnt_sb[:],
                                    ALU.mult)

            psz = P["psW"].tile([H // 2, c.G], F32, tag="wmm")
            nc.tensor.matmul(psz[:], mW1_sb[:], pooled[:],
                             start=True, stop=True)
            zt = P["work"].tile([H // 2, c.G], F32, name="zt", tag="zt")
            nc.vector.tensor_scalar(zt[:], psz[:], mb1_sb[:, 0:1], 0.0,
                                    ALU.add, ALU.max)
            psy = P["psW"].tile([1, c.G], F32, tag="wmm")
            nc.tensor.matmul(psy[:], mW2_sb[:], zt[:], start=True, stop=True)
            y_sb = P["work"].tile([1, c.G], F32, name="y_sb", tag="y_sb")
            nc.vector.tensor_scalar_add(y_sb[:], psy[:], mb2_sb[0:1, 0:1])
            nc.sync.dma_start(out_t.ap().rearrange("g one -> (one) (g)"),
                              y_sb[:])

    nc.compile()
    return nc


# ----------------------------------------------------------------------------
# Host wrapper
# ----------------------------------------------------------------------------

def _fold_bn(conv_b, gamma, beta, mean, var, eps=1e-5):
    s = (gamma * (1.0 / np.sqrt(var + eps))).astype(np.float32)
    t = ((conv_b - mean) * s + beta).astype(np.float32)
    return s, t


def make_in_maps(cfg, prep, inputs):
    c = cfg
    x = np.asarray(inputs["x"], np.float32)
    conv_W = np.asarray(inputs["conv_W"], np.float32)
    conv_b = np.asarray(inputs["conv_b"], np.float32)
    bn_g = np.asarray(inputs["bn_gamma"], np.float32)
    bn_b = np.asarray(inputs["bn_beta"], np.float32)
    bn_m = np.asarray(inputs["bn_mean"], np.float32)
    bn_v = np.asarray(inputs["bn_var"], np.float32)

    iota1 = np.tile(np.arange(128, dtype=np.float32), (128, 1))
    iota2 = iota1 + 128.0
    iotaG = np.tile(np.arange(c.G, dtype=np.float32), (128, 1))
    common = {
        "W_in": np.asarray(inputs["W_in"], np.float32),
        "bin_t": np.tile(np.asarray(inputs["b_in"], np.float32), (128, 1)),
        "iota1": iota1, "iota2": iota2, "iotaG": iotaG,
        "ident": np.eye(128, dtype=np.float32),
        "invcnt_t": np.tile(prep.invcnt, (c.G, 1)),
        "mW1": np.asarray(inputs["mlp_W1"], np.float32),
        "mb1": np.asarray(inputs["mlp_b1"], np.float32).reshape(-1, 1),
        "mW2": np.asarray(inputs["mlp_W2"], np.float32),
        "mb2": np.asarray(inputs["mlp_b2"], np.float32).reshape(1, 1),
    }
    for i in range(2):
        s, t = _fold_bn(conv_b[i], bn_g[i], bn_b[i], bn_m[i], bn_v[i])
        common[f"convW{i}"] = conv_W[i]
        common[f"s_t{i}"] = np.tile(s, (128, 1))
        common[f"t_t{i}"] = np.tile(t, (128, 1))

    in_maps = []
    for k in range(NCORES):
        m = dict(common)
        m["xT"] = x[k * c.NPC:(k + 1) * c.NPC, :].T.copy()
        m["dinv_w"] = prep.dinv_w[k]
        m["batch_w"] = prep.batch_w[k]
        m["gidx"] = prep.wrap16(prep.gidx[k])
        m["dstrel"] = prep.wrap128(prep.dstrel[k])
        in_maps.append(m)
    return in_maps


def run(inputs, cfg=None, trace=False):
    from concourse.bass_utils import run_bass_kernel_spmd
    cfg = cfg or Cfg()
    prep = Prep(cfg, np.asarray(inputs["edge_index"]),
                np.asarray(inputs["batch"]))
    nc = build_kernel(cfg, prep)
    in_maps = make_in_maps(cfg, prep, inputs)
    res = run_bass_kernel_spmd(nc, in_maps, core_ids=list(range(NCORES)),
                               trace=trace)
    return res.results[0]["out"], res


def kernel(**inputs) -> np.ndarray:
    out, _ = run(inputs)
    return out

